# revision 4
# baseline (speedup 1.0000x reference)
"""MixHop layer (3 hops) on 8 Trainium2 NeuronCores.

out = concat_j [ adj_t^j @ (x @ W_j.T + b_j) ]   for j = 0,1,2

Strategy (destination sharding, one SPMD program on 8 cores), optimized to
minimize host<->device traffic over the (slow) axon link AND device time:
  - Hop 0 (y0 = x@W0.T+b0) is a plain dense sgemm with no message passing;
    it is computed on the host in fp32 (also improves accuracy of 1/3 of
    the output).  The device handles everything involving the graph.
  - Each core receives ONLY its own x shard (fp16, transposed): Phase B
    projects the shard -> [NSB, 2F] fp16 ([y1|y2] rows), and an AllGather
    assembles the full gather table [NC*NSB, 2F] on every core.  (The
    baseline shipped the full replicated x to all 8 cores: 8x the bytes
    and 8x the projection work.)
  - Phase C (SpMM1): dma_gather 512B fp16 table rows per in-edge (block-
    major global chunk stream, <=1024 ids per gather), build the
    one-hot*weight segment matrix S on device (tensor_scalar is_equal+mult
    against an iota tile), segment-sum via PE matmuls accumulated in PSUM.
    Cols 0:F -> out1, cols F:2F -> z2 shard; both written with batched
    dma_scatter_add (strided elem_step for the merged out buffer).
  - Phase D: AllGather z2 shards -> full z2 table [NC*NSB, F] fp16.
  - Phase E (SpMM2): same edge structure gathers z2 -> out2.
  - Source-node ids are remapped host-side into the padded concatenated
    table row space (node g -> (g//NS)*NSB + g%NS) so ONE idx encoding
    serves both SpMMs.
  - Transfers: gather/scatter idx streams are shipped compact [16, X] and
    replicated to the 8 GPSIMD core groups on device; (dest,weight) meta
    ships fp16 and is widened on device; iota is generated on device; the
    single fp16 output [NSP, 2F] is scattered in place and fetched once.
    Outputs are pre-zeroed ON DEVICE, so no zero buffers are uploaded
    (custom PJRT runner passes no operands for outputs).
All per-core variation (indices, segment data, scatter rows) is carried as
input data so a single program serves all cores.  fp16 is used only for
gather-table payloads, the segment matrix and the final out buffer (PSUM
accumulation stays fp32); measured end-to-end error vs the fp32 reference
is ~5e-4 max-rel.
"""

import sys

sys.path.insert(0, "/opt/trn_rl_repo")

import heapq

import numpy as np

import concourse.bass as bass
import concourse.tile as tile
from concourse import bacc, mybir

P = 128


class Cfg:
    def __init__(self, n_nodes, n_feat, n_cores, k0max, k1max):
        assert n_nodes % n_cores == 0
        self.N = n_nodes
        self.F = n_feat
        self.NC = n_cores
        self.NS = n_nodes // n_cores          # dests per core
        self.NBLK = -(-self.NS // P)          # blocks per core
        self.NSB = self.NBLK * P              # padded shard rows
        self.NT8 = n_cores * self.NSB         # table rows (concat shards)
        self.K0 = k0max                       # window-0 chunks per block
        self.K1 = k1max                       # window-1 chunks per block
        self.K = k0max + k1max
        self.GMAX = 8                         # chunks per dma_gather (<=1024 ids)
        self.SGRP = 8                         # blocks per dma_scatter_add
        self.NSG = -(-self.NBLK // self.SGRP)
        self.NG0 = -(-(self.NBLK * k0max) // self.GMAX)   # win0 gathers/pass
        self.NG1 = -(-(self.NBLK * k1max) // self.GMAX)
        self.WIN = 32768 if self.NT8 > 32768 else max(P, self.NT8 // 2)


def _balanced_blocks(local_dest, ns, nblk):
    """Assign dests 0..ns-1 to nblk blocks of <=P slots, balancing edge
    counts.  Returns (block_of[ns], pos_of[ns], ids[P, nblk])."""
    deg = np.bincount(local_dest, minlength=ns)
    order = np.argsort(-deg, kind="stable")
    heap = [(0, 0, b) for b in range(nblk)]
    heapq.heapify(heap)
    block_of = np.empty(ns, np.int32)
    pos_of = np.empty(ns, np.int32)
    for d in order:
        while True:
            load, cnt, b = heapq.heappop(heap)
            if cnt < P:
                break
        block_of[d] = b
        pos_of[d] = cnt
        heapq.heappush(heap, (load + int(deg[d]), cnt + 1, b))
    # slot p of block b -> local output row (trash rows ns+p for empty slots)
    ids = np.empty((P, nblk), np.int32)
    for p in range(P):
        ids[p, :] = ns + p
    ids[pos_of, block_of] = np.arange(ns, dtype=np.int32)
    return block_of, pos_of, ids


def _precompute_core(r_loc, c_tab, w, cfg):
    """c_tab: edge source ids already remapped into table row space."""
    ns, nblk = cfg.NS, cfg.NBLK
    block_of, pos_of, ids = _balanced_blocks(r_loc, ns, nblk)
    b_e = block_of[r_loc]
    dl_e = pos_of[r_loc]
    win_e = (c_tab >= cfg.WIN).astype(np.int64)
    order = np.lexsort((np.arange(len(r_loc)), win_e, b_e))
    b_s, win_s, dl_s, c_s, w_s = (
        b_e[order], win_e[order], dl_e[order], c_tab[order], w[order])
    key = b_s * 2 + win_s
    cnt = np.bincount(key, minlength=nblk * 2).reshape(nblk, 2)
    k0need = max(1, int(np.ceil(cnt[:, 0].max() / P))) if len(r_loc) else 1
    k1need = max(1, int(np.ceil(cnt[:, 1].max() / P))) if len(r_loc) else 1
    return dict(b=b_s, win=win_s, dl=dl_s, c=c_s, w=w_s, cnt=cnt, ids=ids,
                k0=k0need, k1=k1need)


def _encode_core(pc, cfg):
    """Device input arrays for one core, given global K0/K1.  Gather and
    scatter id streams are encoded compact [16, cols]; the device
    replicates them to all 8 GPSIMD core groups (128 partitions)."""
    nblk, K0, K1, K = cfg.NBLK, cfg.K0, cfg.K1, cfg.K
    cnt = pc["cnt"]
    idx0 = np.zeros((nblk, K0 * P), np.int16)     # padded edge ids (win0)
    idx1 = np.zeros((nblk, K1 * P), np.int16)
    meta = np.zeros((P, nblk, K, 2), np.float16)  # (local dest, weight)
    starts = np.zeros(nblk * 2, np.int64)
    starts[1:] = np.cumsum(cnt.reshape(-1))[:-1]
    key = pc["b"] * 2 + pc["win"]
    iw = np.arange(len(key)) - starts[key]        # index within (b, win)
    b, win, dl, c, w = pc["b"], pc["win"], pc["dl"], pc["c"], pc["w"]
    m0 = win == 0
    idx0[b[m0], iw[m0]] = c[m0].astype(np.int16)
    m1 = ~m0
    idx1[b[m1], iw[m1]] = (c[m1] - cfg.WIN).astype(np.int16)
    kk = np.where(m0, iw // P, K0 + iw // P)
    meta[iw % P, b, kk, 0] = dl
    meta[iw % P, b, kk, 1] = w

    # compact chunk-stream gather encodings [16, n_gath*GMAX*8]; dma_gather
    # reads logical id i from [i%16, i//16] of its idx window (device
    # replicates to all 8 core groups).
    GM = cfg.GMAX

    def enc(idx, Kw, n_gath):
        stream = idx.reshape(nblk * Kw * P)
        out = np.zeros((16, n_gath * GM * 8), np.int16)
        for g in range(n_gath):
            cg = min(GM, nblk * Kw - GM * g)
            flat = stream[g * GM * P: g * GM * P + cg * P]
            out[:, g * GM * 8: g * GM * 8 + cg * 8] = flat.reshape(-1, 16).T
        return out

    # batched scatter ids: group g covers SGRP blocks; logical i = c*128+p
    ids = pc["ids"]
    sid = np.zeros((16, cfg.NSG * cfg.SGRP * 8), np.int16)
    for g in range(cfg.NSG):
        nb = min(cfg.SGRP, nblk - g * cfg.SGRP)
        flat = ids[:, g * cfg.SGRP: g * cfg.SGRP + nb].T.reshape(-1)
        sid[:, g * cfg.SGRP * 8: g * cfg.SGRP * 8 + nb * 8] = (
            flat.reshape(-1, 16).T.astype(np.int16))
    idxp = np.concatenate(
        [enc(idx0, K0, cfg.NG0), enc(idx1, K1, cfg.NG1), sid], axis=1)
    return dict(
        idxp=np.ascontiguousarray(idxp),
        meta16=np.ascontiguousarray(meta.reshape(P, nblk * K * 2)),
    )


def _build_program(cfg, phases="BCDE"):
    F, NC = cfg.F, cfg.NC
    NS, NBLK, K0, K1, K = cfg.NS, cfg.NBLK, cfg.K0, cfg.K1, cfg.K
    NSB, NT8 = cfg.NSB, cfg.NT8
    NW0 = min(NT8, cfg.WIN)
    NSP = NS + P                             # out buf rows incl trash
    f32 = mybir.dt.float32
    f16 = mybir.dt.float16
    i16 = mybir.dt.int16
    GM, NG0, NG1 = cfg.GMAX, cfg.NG0, cfg.NG1
    SG, NSG = cfg.SGRP, cfg.NSG
    L0 = NG0 * GM * 8
    L1 = NG1 * GM * 8
    LS = NSG * SG * 8

    nc = bacc.Bacc("TRN2", target_bir_lowering=False, debug=False,
                   enable_asserts=False, num_devices=NC, num_swdge_queues=4)

    # ---- inputs ----------------------------------------------------------
    xsT_in = nc.dram_tensor("xsT", [F, NSB], f16, kind="ExternalInput").ap()
    wb_in = nc.dram_tensor("wb", [2 * F + 2, F], f16,
                           kind="ExternalInput").ap()
    idxp_in = nc.dram_tensor("idxp", [16, L0 + L1 + LS], i16,
                             kind="ExternalInput").ap()
    meta_in = nc.dram_tensor("meta16", [P, NBLK * K * 2], f16,
                             kind="ExternalInput").ap()

    # ---- outputs / scratch ----------------------------------------------
    out_buf = nc.dram_tensor("out", [NSP, 2 * F], f16,
                             kind="ExternalOutput").ap()
    tsh = nc.dram_tensor("tsh", [NSB, 2 * F], f16, kind="Internal").ap()
    table = nc.dram_tensor("table", [NT8, 2 * F], f16, kind="Internal",
                           addr_space="Shared").ap()
    z2s = nc.dram_tensor("z2s", [NSP, F], f16, kind="Internal").ap()
    z2t = nc.dram_tensor("z2t", [NT8, F], f16, kind="Internal",
                         addr_space="Shared").ap()

    with tile.TileContext(nc) as tc:
        with tc.tile_pool(name="const", bufs=1) as cpool:
            # iota (generated on device; values 0..127 are exact in f32)
            iota_t = cpool.tile([P, P], f32)
            nc.gpsimd.iota(iota_t[:], pattern=[[1, P]], base=0,
                           channel_multiplier=0,
                           allow_small_or_imprecise_dtypes=True)
            # meta fp16 -> fp32 (tensor_scalar scalars must match in0 dtype)
            meta16_t = cpool.tile([P, NBLK * K * 2], f16)
            nc.sync.dma_start(meta16_t[:], meta_in[:])
            meta_t = cpool.tile([P, NBLK * K * 2], f32)
            nc.vector.tensor_copy(meta_t[:], meta16_t[:])
            # idx streams: compact [16, X] in DRAM, replicate to the 8
            # GPSIMD core groups (partition groups of 16)
            ix0_t = cpool.tile([P, L0], i16)
            ix1_t = cpool.tile([P, L1], i16)
            sid_t = cpool.tile([P, LS], i16)
            for g8 in range(8):
                pr = slice(g8 * 16, (g8 + 1) * 16)
                nc.sync.dma_start(ix0_t[pr, :], idxp_in[:, 0:L0])
                nc.sync.dma_start(ix1_t[pr, :], idxp_in[:, L0:L0 + L1])
                nc.sync.dma_start(sid_t[pr, :], idxp_in[:, L0 + L1:])
            # weights / biases
            wt_t = []
            b16_t = []
            for j in range(2):
                wtj = cpool.tile([F, F], f16, tag=f"wt{j}", name=f"wt{j}")
                nc.sync.dma_start(wtj[:], wb_in[j * F:(j + 1) * F, :])
                wt_t.append(wtj)
                b16j = cpool.tile([1, F], f16, tag=f"b16{j}", name=f"b16{j}")
                nc.sync.dma_start(b16j[:], wb_in[2 * F + j:2 * F + j + 1, :])
                b16_t.append(b16j)
            ones_t = cpool.tile([1, P], f16)
            nc.vector.memset(ones_t[:], 1.0)
            # own x shard, transposed; kept resident for Phase B
            xs_t = cpool.tile([F, NSB], f16)
            nc.sync.dma_start(xs_t[:], xsT_in[:])

            # ---- zero scatter-add bases (z2s and out) on device ----------
            if "C" in phases or "E" in phases:
                with tc.tile_pool(name="zz", bufs=1) as zpool:
                    zt = zpool.tile([P, 2048], f16)
                    nc.vector.memset(zt[:], 0.0)
                    nrow = 0
                    while nrow + 2048 <= NSP:
                        nc.sync.dma_start(
                            z2s[nrow:nrow + 2048, :].rearrange(
                                "(a b) f -> a (b f)", a=P), zt[:])
                        nrow += 2048
                    while nrow + P <= NSP:
                        nc.sync.dma_start(
                            z2s[nrow:nrow + P, :].rearrange(
                                "(a b) f -> a (b f)", a=P), zt[:, :F])
                        nrow += P
                    assert nrow >= NSB, (nrow, NSB)
                    nrow = 0
                    while nrow + 1024 <= NSP:
                        nc.sync.dma_start(
                            out_buf[nrow:nrow + 1024, :].rearrange(
                                "(a b) f -> a (b f)", a=P), zt[:])
                        nrow += 1024
                    while nrow + P <= NSP:
                        nc.sync.dma_start(
                            out_buf[nrow:nrow + P, :].rearrange(
                                "(a b) f -> a (b f)", a=P), zt[:, :2 * F])
                        nrow += P
                    assert nrow >= NS, (nrow, NS)

            # ---- Phase B: project own shard -> [NSB, 2F] fp16 ------------
            # 512-node groups: 4x2 matmuls (+rank-1 bias matmul) from the
            # resident x tile, one wide store.  psum->staging copies
            # alternate DVE/ACT to spread engine load.
            if "B" in phases:
             with tc.tile_pool(name="projB", bufs=3) as bpool, \
                  tc.tile_pool(name="psumB", bufs=4, space="PSUM") as bpsum:
                for t in range(-(-NSB // 512)):
                    r0 = t * 512
                    gw = min(512, NSB - r0)
                    st = bpool.tile([P, 4, 2 * F], f16, tag="stb")
                    nsub = -(-gw // P)
                    for s in range(nsub):
                        c0 = r0 + s * P
                        ps = bpsum.tile([P, 2 * F], f32, space="PSUM")
                        for j in range(2):
                            nc.tensor.matmul(
                                ps[:, j * F:(j + 1) * F],
                                lhsT=xs_t[:, c0:c0 + P], rhs=wt_t[j][:],
                                start=True, stop=False)
                            nc.tensor.matmul(
                                ps[:, j * F:(j + 1) * F],
                                lhsT=ones_t[:],
                                rhs=b16_t[j][:],
                                start=False, stop=True)
                        if (t + s) % 2 == 0:
                            nc.vector.tensor_copy(st[:, s, :], ps[:])
                        else:
                            nc.scalar.copy(st[:, s, :], ps[:])
                    nc.sync.dma_start(
                        tsh[r0:r0 + gw, :].rearrange("(b a) f -> a b f", a=P),
                        st[:, :nsub, :])

            # ---- Phase G: AllGather table shards -------------------------
            if "B" in phases:
                nc.gpsimd.collective_compute(
                    "AllGather", mybir.AluOpType.bypass,
                    replica_groups=[list(range(NC))],
                    ins=[tsh[:]], outs=[table[:]],
                )

            # ---- SpMM machinery ------------------------------------------
            def spmm(src_w0, src_w1, fdim, dst_bufs):
                """Gathers stream GM-chunk slices of the global block-major
                chunk stream per window; segment matmuls accumulate per
                block in PSUM; batched scatter-add to pre-zeroed buffers.
                dst_bufs: list of (dst_ap, psum col offset, elem_step)."""
                with tc.tile_pool(name="ga", bufs=4) as gapool, \
                     tc.tile_pool(name="sS", bufs=4) as spool, \
                     tc.tile_pool(name="stg", bufs=2) as stgpool, \
                     tc.tile_pool(name="psC", bufs=4, space="PSUM") as cpsum:
                    wins = [[src_w0, ix0_t, NBLK * K0, [], 0],
                            [src_w1, ix1_t, NBLK * K1, [], 0]]

                    def ensure_gathers(w, upto_chunk):
                        src_w, ix_t, tot, tiles, _ = wins[w]
                        while wins[w][4] * GM < min(upto_chunk, tot):
                            g = wins[w][4]
                            cg = min(GM, tot - GM * g)
                            ga = gapool.tile([P, GM, fdim], f16,
                                             tag=f"ga{w}", name=f"ga{w}_{g}")
                            nc.gpsimd.dma_gather(
                                ga[:, :cg, :], src_w,
                                ix_t[:, g * GM * 8: g * GM * 8 + cg * 8],
                                num_idxs=cg * P, num_idxs_reg=cg * P,
                                elem_size=fdim, queue_num=0)
                            tiles.append(ga)
                            wins[w][4] += 1

                    stgs = None
                    for b in range(NBLK):
                        g_s, c_s = b // SG, b % SG
                        nb = min(SG, NBLK - g_s * SG)
                        if c_s == 0:
                            stgs = [stgpool.tile([P, SG, F], f16,
                                                 tag=f"stg{i}",
                                                 name=f"stg{i}_{g_s}")
                                    for i in range(len(dst_bufs))]
                        ensure_gathers(0, (b + 1) * K0)
                        ensure_gathers(1, (b + 1) * K1)
                        ps = cpsum.tile([P, fdim], f32, space="PSUM")
                        for k in range(K):
                            S = spool.tile([P, P], f16, tag="S")
                            mo = (b * K + k) * 2
                            nc.vector.tensor_scalar(
                                out=S[:], in0=iota_t[:],
                                scalar1=meta_t[:, mo:mo + 1],
                                scalar2=meta_t[:, mo + 1:mo + 2],
                                op0=mybir.AluOpType.is_equal,
                                op1=mybir.AluOpType.mult)
                            if k < K0:
                                gk = b * K0 + k
                                rhs = wins[0][3][gk // GM][:, gk % GM, :]
                            else:
                                gk = b * K1 + (k - K0)
                                rhs = wins[1][3][gk // GM][:, gk % GM, :]
                            nc.tensor.matmul(ps[:], lhsT=S[:], rhs=rhs,
                                             start=(k == 0),
                                             stop=(k == K - 1))
                        for i, (dst, coff, estep) in enumerate(dst_bufs):
                            nc.vector.tensor_copy(stgs[i][:, c_s, :],
                                                  ps[:, coff:coff + F])
                        if c_s == nb - 1:
                            for i, (dst, coff, estep) in enumerate(dst_bufs):
                                nc.gpsimd.dma_scatter_add(
                                    dst, stgs[i][:, :nb, :],
                                    sid_t[:, g_s * SG * 8:
                                          g_s * SG * 8 + nb * 8],
                                    num_idxs=nb * P, num_idxs_reg=nb * P,
                                    elem_size=F, elem_step=estep,
                                    queue_num=0)

            # ---- Phase C: SpMM1 over table -> out1, z2s ------------------
            if "C" in phases:
                spmm(table[:NW0, :], table[cfg.WIN:NT8, :], 2 * F,
                     [(out_buf[:, 0:F], 0, 2 * F), (z2s[:], F, None)])

            # ---- Phase D: AllGather z2 shards ----------------------------
            if "D" in phases:
                nc.gpsimd.collective_compute(
                    "AllGather", mybir.AluOpType.bypass,
                    replica_groups=[list(range(NC))],
                    ins=[z2s[0:NSB, :]], outs=[z2t[:]],
                )

            # ---- Phase E: SpMM2 over z2 table -> out2 --------------------
            if "E" in phases:
                spmm(z2t[:NW0, :], z2t[cfg.WIN:NT8, :], F,
                     [(out_buf[:, F:2 * F], 0, 2 * F)])

    nc.compile()
    return nc


_CACHE = {}


def _get_program(cfg, phases="BCDE"):
    key = (cfg.N, cfg.F, cfg.NC, cfg.K0, cfg.K1, phases)
    if key not in _CACHE:
        _CACHE[key] = _build_program(cfg, phases)
    return _CACHE[key]


_RUNNERS = {}


def _get_runner(nc, n_cores):
    """Minimal PJRT runner for a prebuilt SPMD Bass program.  Unlike
    bass_utils.run_bass_kernel_spmd it does NOT upload zero buffers for
    the outputs (this kernel zeroes its outputs on device), which saves
    one full output-sized host->device transfer over the slow axon link."""
    key = id(nc)
    if key in _RUNNERS:
        return _RUNNERS[key]
    import jax
    from jax.sharding import Mesh, PartitionSpec
    from jax.experimental.shard_map import shard_map
    from concourse import bass2jax as b2j

    b2j.install_neuronx_cc_hook()
    partition_name = (nc.partition_id_tensor.name
                      if nc.partition_id_tensor else None)
    in_names, out_names, out_avals = [], [], []
    for alloc in nc.m.functions[0].allocations:
        if not isinstance(alloc, mybir.MemoryLocationSet):
            continue
        name = alloc.memorylocations[0].name
        if alloc.kind == "ExternalInput":
            if name != partition_name:
                in_names.append(name)
        elif alloc.kind == "ExternalOutput":
            out_names.append(name)
            out_avals.append(jax.core.ShapedArray(
                tuple(alloc.tensor_shape), mybir.dt.np(alloc.dtype)))
    n_params = len(in_names)
    param_names = list(in_names)
    if partition_name is not None:
        in_names.append(partition_name)

    def _body(*args):
        operands = list(args)
        if partition_name is not None:
            operands.append(b2j.partition_id_tensor())
        outs = b2j._bass_exec_p.bind(
            *operands,
            out_avals=tuple(out_avals),
            in_names=tuple(in_names),
            out_names=tuple(out_names),
            lowering_input_output_aliases=(),
            sim_require_finite=True,
            sim_require_nnan=True,
            nc=nc,
        )
        return tuple(outs)

    devices = jax.devices()[:n_cores]
    assert len(devices) == n_cores, (
        f"need {n_cores} devices, only {len(jax.devices())} visible")
    mesh = Mesh(np.asarray(devices), ("core",))
    sharded = jax.jit(shard_map(
        _body, mesh=mesh,
        in_specs=(PartitionSpec("core"),) * n_params,
        out_specs=(PartitionSpec("core"),) * len(out_names),
        check_rep=False))

    def run(in_maps):
        concat_in = [
            np.concatenate([np.asarray(m[name]) for m in in_maps], axis=0)
            for name in param_names
        ]
        out_arrs = sharded(*concat_in)
        return [
            {name: np.asarray(out_arrs[i]).reshape(
                n_cores, *out_avals[i].shape)[c]
             for i, name in enumerate(out_names)}
            for c in range(n_cores)
        ]

    _RUNNERS[key] = run
    return run


def _prepare(x, edge_weight, W, b, row, col, n_cores=8):
    N, F = np.asarray(x).shape
    row = np.asarray(row).astype(np.int64)
    col = np.asarray(col).astype(np.int64)
    w = np.asarray(edge_weight).astype(np.float32)
    x = np.asarray(x).astype(np.float32)
    W = np.asarray(W).astype(np.float32)
    b = np.asarray(b).astype(np.float32)

    ns = N // n_cores
    core_of = row // ns
    cfg0 = Cfg(N, F, n_cores, 1, 1)
    # remap source node ids into the padded concatenated table row space
    c_tab = (col // ns) * cfg0.NSB + (col % ns)
    pcs = []
    for m in range(n_cores):
        sel = np.where(core_of == m)[0]
        pcs.append(_precompute_core(row[sel] - m * ns, c_tab[sel], w[sel],
                                    cfg0))
    k0 = max(pc["k0"] for pc in pcs)
    k1 = max(pc["k1"] for pc in pcs)
    cfg = Cfg(N, F, n_cores, k0, k1)

    wb = np.concatenate(
        [W[1].T, W[2].T, b[1:2, :], b[2:3, :]], axis=0).astype(np.float16)
    in_maps = []
    for m in range(n_cores):
        enc = _encode_core(pcs[m], cfg)
        xs = np.zeros((F, cfg.NSB), np.float16)
        xs[:, :ns] = x[m * ns:(m + 1) * ns, :].T.astype(np.float16)
        in_maps.append(dict(
            xsT=xs, wb=wb, idxp=enc["idxp"], meta16=enc["meta16"],
        ))
    return cfg, in_maps


def kernel(x, edge_weight, W, b, row, col):
    n_cores = 8
    x = np.asarray(x).astype(np.float32)
    W = np.asarray(W).astype(np.float32)
    b = np.asarray(b).astype(np.float32)
    N, F = x.shape
    ns = N // n_cores
    cfg, in_maps = _prepare(x, edge_weight, W, b, row, col, n_cores)
    nc = _get_program(cfg)
    run = _get_runner(nc, n_cores)
    res = run(in_maps)
    y0 = x @ W[0].T + b[0]                     # hop 0: dense, fp32 on host
    outs = []
    for m in range(n_cores):
        o = res[m]["out"][:ns].astype(np.float32)
        outs.append(np.concatenate(
            [y0[m * ns:(m + 1) * ns], o[:, :F], o[:, F:]], axis=1))
    return np.concatenate(outs, axis=0)
